# revision 6
# baseline (speedup 1.0000x reference)
import sys

sys.path.insert(0, '/opt/trn_rl_repo')

import ml_dtypes
import numpy as np
import concourse.bass as bass
import concourse.mybir as mybir
import concourse.tile as tile
from concourse import bacc, bass_utils

F32 = mybir.dt.float32
F32R = mybir.dt.float32r
BF16 = mybir.dt.bfloat16
FP16 = mybir.dt.float16
AF = mybir.ActivationFunctionType

D_MODEL = 1024
N_HEADS = 16
D_HEAD = 64
SEQ = 2048
BATCH = 2
N_CORES = 8
HPC = 4             # heads per core
CSL = HPC * D_HEAD  # 256: qkv feature slice per core
NT_D = D_MODEL // 128  # 8
NQ = SEQ // 512     # 4 q-chunks
GROUPS = [[0, 1, 2, 3], [4, 5, 6, 7]]

_cache = {}


def _build():
    nc = bacc.Bacc("TRN2", target_bir_lowering=False, debug=False,
                   num_devices=N_CORES)
    xt_in = nc.dram_tensor("xt", [D_MODEL, SEQ], FP16, kind="ExternalInput").ap()
    wq_in = nc.dram_tensor("wq", [128, NT_D * CSL], FP16, kind="ExternalInput").ap()
    wk_in = nc.dram_tensor("wk", [128, NT_D * CSL], FP16, kind="ExternalInput").ap()
    wv_in = nc.dram_tensor("wv", [128, NT_D * CSL], FP16, kind="ExternalInput").ap()
    wo_in = nc.dram_tensor("wo", [128, 2 * D_MODEL], FP16, kind="ExternalInput").ap()
    bqk_in = nc.dram_tensor("bqk", [128, 4], F32, kind="ExternalInput").ap()
    bv_in = nc.dram_tensor("bv", [1, CSL], FP16, kind="ExternalInput").ap()
    bo4_in = nc.dram_tensor("bo4", [128, D_MODEL], F32, kind="ExternalInput").ap()
    nm_in = nc.dram_tensor("negm2", [128, 256], F32, kind="ExternalInput").ap()
    o16_in = nc.dram_tensor("ones16", [1, 128], FP16, kind="ExternalInput").ap()
    or_in = nc.dram_tensor("onesr", [1, 128], F32, kind="ExternalInput").ap()
    out = nc.dram_tensor("out", [NQ * 128, D_MODEL], FP16,
                         kind="ExternalOutput").ap()

    with tile.TileContext(nc) as tc:
        _body(nc, tc, xt_in, wq_in, wk_in, wv_in, wo_in, bqk_in, bv_in,
              bo4_in, nm_in, o16_in, or_in, out)
    nc.compile()
    return nc


def _body(nc, tc, xt_in, wq_in, wk_in, wv_in, wo_in, bqk_in, bv_in,
          bo4_in, nm_in, o16_in, or_in, out):
    from contextlib import ExitStack
    ctx = ExitStack()
    with ctx:
        const = ctx.enter_context(tc.tile_pool(name="const", bufs=1))
        wpool = ctx.enter_context(tc.tile_pool(name="wpool", bufs=1))
        xtpool = ctx.enter_context(tc.tile_pool(name="xtpool", bufs=1))
        qkpool = ctx.enter_context(tc.tile_pool(name="qkpool", bufs=1))
        vpool = ctx.enter_context(tc.tile_pool(name="vpool", bufs=1))
        htpool = ctx.enter_context(tc.tile_pool(name="htpool", bufs=1))
        exp_pool = ctx.enter_context(tc.tile_pool(name="exp_pool", bufs=3))
        misc_c = ctx.enter_context(tc.tile_pool(name="misc_c", bufs=2))
        stpool = ctx.enter_context(tc.tile_pool(name="stpool", bufs=2))
        drpool = ctx.enter_context(tc.tile_pool(name="drpool", bufs=1, space="DRAM"))
        ps_pair = ctx.enter_context(tc.tile_pool(name="ps_pair", bufs=2, space="PSUM"))
        ps_att = ctx.enter_context(tc.tile_pool(name="ps_att", bufs=1, space="PSUM"))
        ps_m = ctx.enter_context(tc.tile_pool(name="ps_m", bufs=2, space="PSUM"))

        # ---- x^T tiles: chunk-0 columns first (startup critical path) ----
        xT = [xtpool.tile([128, SEQ], FP16, name=f"xT{dj}", tag=f"xT{dj}")
              for dj in range(NT_D)]
        for dj in range(NT_D):
            nc.sync.dma_start(xT[dj][:, 0:512],
                              xt_in[128 * dj:128 * (dj + 1), 0:512])

        # ---- weights (straight copies; host pre-packed) ----
        wq_t = wpool.tile([128, NT_D * CSL], FP16, name="w_wq")
        nc.scalar.dma_start(wq_t[:], wq_in[:])
        wk_t = wpool.tile([128, NT_D * CSL], FP16, name="w_wk")
        nc.scalar.dma_start(wk_t[:], wk_in[:])
        wv_t = wpool.tile([128, NT_D * CSL], FP16, name="w_wv")
        nc.scalar.dma_start(wv_t[:], wv_in[:])
        wo_t = wpool.tile([128, 2 * D_MODEL], FP16, name="w_wo")
        nc.scalar.dma_start(wo_t[:], wo_in[:])

        # ---- consts ----
        negm2 = const.tile([128, 256], F32)
        nc.scalar.dma_start(negm2[:], nm_in[:])
        bqk = const.tile([128, 4], F32)
        nc.scalar.dma_start(bqk[:], bqk_in[:])
        bv_t = const.tile([1, CSL], FP16)
        nc.scalar.dma_start(bv_t[:], bv_in[:])
        bo4 = const.tile([128, D_MODEL], F32)
        nc.scalar.dma_start(bo4[:], bo4_in[:])
        ones16 = const.tile([1, 128], FP16)
        nc.scalar.dma_start(ones16[:], o16_in[:])
        onesr = const.tile([1, 128], F32R)
        nc.scalar.dma_start(onesr[:], or_in.bitcast(F32R))

        # ---- rest of x^T columns ----
        for dj in range(NT_D):
            nc.sync.dma_start(xT[dj][:, 512:SEQ],
                              xt_in[128 * dj:128 * (dj + 1), 512:SEQ])

        # ---- persistent activations ----
        qt, kt, hT = [], [], []
        for ci in range(2):
            qt.append(qkpool.tile([128, SEQ], FP16, name=f"qt{ci}", tag=f"qt{ci}"))
            kt.append(qkpool.tile([128, SEQ], FP16, name=f"kt{ci}", tag=f"kt{ci}"))
            hT.append(htpool.tile([128, SEQ], FP16, name=f"hT{ci}", tag=f"hT{ci}"))
        vt = [vpool.tile([128, HPC * 65], FP16, name=f"vt{si}", tag=f"vt{si}")
              for si in range(16)]

        def emit_qk(sj, ci, w_t, bcol, dst):
            """One projection slice: 8 matmuls + ACT bias-add into dst."""
            pp = ps_m.tile([128, 512], F32, name="pp", tag="m")
            for dj in range(NT_D):
                nc.tensor.matmul(
                    pp[:],
                    w_t[:, dj * CSL + 128 * ci:dj * CSL + 128 * (ci + 1)],
                    xT[dj][:, 512 * sj:512 * (sj + 1)],
                    start=(dj == 0), stop=(dj == NT_D - 1))
            nc.scalar.add(dst[ci][:, 512 * sj:512 * (sj + 1)], pp[:],
                          bqk[:, bcol + ci:bcol + ci + 1])

        def emit_v(sj, sl):
            """v for s-tile si: 9 matmuls + ones memset + ACT copy."""
            si = 4 * sj + sl
            pv = ps_m.tile([128, 512], F32, name="pv", tag="m")
            for dj in range(NT_D):
                nc.tensor.matmul(
                    pv[:, 0:CSL],
                    xT[dj][:, 128 * si:128 * (si + 1)],
                    wv_t[:, dj * CSL:(dj + 1) * CSL],
                    start=(dj == 0), stop=False)
            nc.tensor.matmul(pv[:, 0:CSL], ones16[:], bv_t[:],
                             start=False, stop=True)
            nc.vector.memset(
                vt[si].rearrange("p (h e) -> p h e", e=65)[:, :, 64:65], 1.0)
            nc.scalar.copy(
                vt[si].rearrange("p (h e) -> p h e", e=65)[:, :, 0:64],
                pv[:, 0:CSL].rearrange("p (h e) -> p h e", e=64))

        pending = []

        def pop_pending(n):
            for _ in range(min(n, len(pending))):
                pending.pop(0)()

        def emit_attention(sj):
            nk = 4 * sj + 4
            for ci in range(2):
                pa = [ps_att.tile([65, 512], F32, name=f"pa{hh}", tag=f"att{hh}")
                      for hh in range(2)]
                for ki in range(nk):
                    r = ki - 4 * sj
                    c0 = 0 if r < 0 else 128 * r
                    ps = ps_pair.tile([128, 1024], F32, name="ps", tag="sp")
                    for hh in range(2):
                        p0 = 64 * hh
                        nc.tensor.matmul(
                            ps[:, 512 * hh + c0:512 * (hh + 1)],
                            kt[ci][p0:p0 + 64, 128 * ki:128 * (ki + 1)],
                            qt[ci][p0:p0 + 64, 512 * sj + c0:512 * (sj + 1)],
                            start=True, stop=True)
                    if r >= 0:
                        psm = ps.rearrange("p (h c) -> p h c", h=2)[:, :, c0:c0 + 128]
                        nc.vector.tensor_add(
                            psm, psm,
                            negm2.rearrange("p (h c) -> p h c", h=2))
                    et = exp_pool.tile([128, 1024], FP16, name="et", tag="et")
                    nc.scalar.activation(
                        et.rearrange("p (h c) -> p h c", h=2)[:, :, c0:512],
                        ps.rearrange("p (h c) -> p h c", h=2)[:, :, c0:512],
                        AF.Exp)
                    for hh in range(2):
                        h_local = 2 * ci + hh
                        nc.tensor.matmul(
                            pa[hh][:, c0:512],
                            vt[ki][:, 65 * h_local:65 * h_local + 65],
                            et[:, 512 * hh + c0:512 * (hh + 1)],
                            start=(ki == 0), stop=(ki == nk - 1),
                            skip_group_check=True)
                    if ki % 4 == 3:
                        pop_pending(1)
                # softmax denominators -> reciprocal -> broadcast -> scale
                rt = misc_c.tile([1, 1024], FP16, name="rt", tag="rt")
                with nc.allow_low_precision(reason="softmax 1/denom in fp16"):
                    nc.vector.reciprocal(rt[:, 0:512], pa[0][64:65, :])
                    nc.vector.reciprocal(rt[:, 512:1024], pa[1][64:65, :])
                pb = ps_m.tile([128, 512], F32, name="pb", tag="m")
                nc.tensor.matmul(pb[0:64, :], ones16[0:1, 0:64], rt[:, 0:512],
                                 start=True, stop=True)
                nc.tensor.matmul(pb[64:128, :], ones16[0:1, 0:64], rt[:, 512:1024],
                                 start=True, stop=True, tile_position=(0, 64))
                bc = misc_c.tile([128, 512], FP16, name="bc", tag="bc")
                nc.scalar.copy(bc[:], pb[:])
                for hh in range(2):
                    nc.vector.tensor_mul(
                        hT[ci][64 * hh:64 * (hh + 1), 512 * sj:512 * (sj + 1)],
                        pa[hh][0:64, :], bc[64 * hh:64 * (hh + 1), :])
                pop_pending(1)

        def emit_wo(sj):
            """Partial out = hT @ Wo_local + bo/4, fp16, reduce-scattered."""
            pstage = stpool.tile([128, 4 * D_MODEL], FP16, name=f"pst{sj}",
                                 tag="pst")
            for sl in range(4):
                s0 = 512 * sj + 128 * sl
                for h in range(2):
                    po = ps_m.tile([128, 512], F32, name="po", tag="m")
                    for ci in range(2):
                        nc.tensor.matmul(
                            po[:],
                            hT[ci][:, s0:s0 + 128],
                            wo_t[:, ci * D_MODEL + 512 * h:ci * D_MODEL + 512 * (h + 1)],
                            start=(ci == 0), stop=(ci == 1))
                    dst = pstage[:, D_MODEL * sl + 512 * h:D_MODEL * sl + 512 * (h + 1)]
                    nc.vector.tensor_add(dst, po[:], bo4[:, 512 * h:512 * (h + 1)])
                if sl % 2 == 1:
                    pop_pending(1)
            prs = drpool.tile([512, D_MODEL], FP16, name=f"prs{sj}", tag=f"prs{sj}")
            nc.sync.dma_start(
                prs.rearrange("(s p) c -> p s c", p=128),
                pstage.rearrange("p (s c) -> p s c", c=D_MODEL))
            pro = drpool.tile([128, D_MODEL], FP16, name=f"pro{sj}", tag=f"pro{sj}")
            nc.gpsimd.collective_compute(
                "ReduceScatter", mybir.AluOpType.add, replica_groups=GROUPS,
                ins=[prs[:]], outs=[pro[:]])
            nc.sync.dma_start(out[128 * sj:128 * (sj + 1), :], pro[:])

        # ---- pipeline over q-chunks ----
        for ci in range(2):
            emit_qk(0, ci, wq_t, 0, qt)
            emit_qk(0, ci, wk_t, 2, kt)
        for sl in range(4):
            emit_v(0, sl)
        for sj in range(NQ):
            if sj + 1 < NQ:
                nsj = sj + 1
                for ci in range(2):
                    pending.append(lambda c=ci, s=nsj: emit_qk(s, c, wq_t, 0, qt))
                    pending.append(lambda c=ci, s=nsj: emit_qk(s, c, wk_t, 2, kt))
                for sl in range(4):
                    pending.append(lambda s=nsj, l=sl: emit_v(s, l))
            emit_attention(sj)
            emit_wo(sj)
            pop_pending(len(pending))


def _consts():
    kk = np.arange(128)[:, None]
    qq = np.arange(128)[None, :]
    negmask = np.where(kk <= qq, 0.0, -1e30).astype(np.float32)
    negm2 = np.concatenate([negmask, negmask], axis=1)
    return negm2


def _pack_w(w):
    """[1024, C] -> [128, 8*C] fp16 with d-tile t at column block t."""
    c = w.shape[1]
    return np.ascontiguousarray(
        w.reshape(NT_D, 128, c).transpose(1, 0, 2).reshape(128, NT_D * c)
    ).astype(np.float16)


def kernel(x, Wq, bq, Wk, bk, Wv, bv, Wo, bo):
    x = np.asarray(x, dtype=np.float32)
    Wq = np.asarray(Wq, dtype=np.float32)
    bq = np.asarray(bq, dtype=np.float32)
    Wk = np.asarray(Wk, dtype=np.float32)
    bk = np.asarray(bk, dtype=np.float32)
    Wv = np.asarray(Wv, dtype=np.float32)
    bv = np.asarray(bv, dtype=np.float32)
    Wo = np.asarray(Wo, dtype=np.float32)
    bo = np.asarray(bo, dtype=np.float32)

    if "nc" not in _cache:
        _cache["nc"] = _build()
    nc = _cache["nc"]

    negm2 = _consts()
    scale = 1.0 / np.sqrt(np.float32(D_HEAD))
    ones16 = np.ones((1, 128), dtype=np.float16)
    onesr = np.ones((1, 128), dtype=np.float32)
    bo4 = np.ascontiguousarray(
        np.broadcast_to((bo / 4.0)[None, :], (128, D_MODEL))).astype(np.float32)
    in_maps = []
    for core in range(N_CORES):
        b, g = divmod(core, HPC)
        csl = slice(CSL * g, CSL * (g + 1))
        wo_loc = Wo[csl, :]  # [256, 1024]
        wo_pack = np.ascontiguousarray(
            wo_loc.reshape(2, 128, D_MODEL).transpose(1, 0, 2).reshape(
                128, 2 * D_MODEL)).astype(np.float16)
        bqk = np.stack([
            bq[csl][0:128] * scale, bq[csl][128:256] * scale,
            bk[csl][0:128], bk[csl][128:256],
        ], axis=1).astype(np.float32)
        in_maps.append({
            "xt": np.ascontiguousarray(x[b].T).astype(np.float16),
            "wq": _pack_w(Wq[:, csl] * scale),
            "wk": _pack_w(Wk[:, csl]),
            "wv": _pack_w(Wv[:, csl]),
            "wo": wo_pack,
            "bqk": np.ascontiguousarray(bqk),
            "bv": np.ascontiguousarray(bv[None, csl]).astype(np.float16),
            "bo4": bo4,
            "negm2": negm2,
            "ones16": ones16,
            "onesr": onesr,
        })

    # the axon terminal occasionally reports a transient
    # NRT_EXEC_UNIT_UNRECOVERABLE; retries with backoff recover it
    import time as _time
    for attempt in range(3):
        try:
            res = bass_utils.run_bass_kernel_spmd(
                nc, in_maps, core_ids=list(range(N_CORES)))
            break
        except Exception:
            if attempt == 2:
                raise
            _time.sleep(5.0 * (attempt + 1))

    full = np.empty((BATCH, SEQ, D_MODEL), dtype=np.float32)
    for core in range(N_CORES):
        b, g = divmod(core, HPC)
        o = np.asarray(res.results[core]["out"]).astype(np.float32)
        for qi in range(NQ):
            full[b, 512 * qi + 128 * g:512 * qi + 128 * (g + 1), :] = \
                o[128 * qi:128 * (qi + 1), :]
    return full
